# revision 2
# baseline (speedup 1.0000x reference)
"""Trainium2 Bass kernel for nn_CrossEntGroup.

Reference computation (see problem):
    labels = target_labels - 1                      # -1 => ignored
    per class c: mask rows with label==c, col_sum S[c,g], p = Am/S,
    M[c,i,j] = sum_n p[n,i] log p[n,j],  loss = mean over valid classes of
    sum_{i!=j} M[c,i,j] / (G*(G-1))

Algebraic reduction used here (single pass over the data):
    sel[n,:]  = group_act[label[n], n, :]       (selected row)
    L[n]      = sum_j log sel[n,j]
    S[c,i]    = sum_{n in c} sel[n,i]
    B[c,i]    = sum_{n in c} sel[n,i] * (L[n] - log sel[n,i])
    per_class[c] = sum_i B[c,i]/S[c,i] - (G-1) * sum_i log S[c,i]
    out = sum_valid per_class / (n_valid * G * (G-1))

Device strategy (per core, N sharded 8 ways -> NS=62500 samples):
  * samples laid out [P=125 partitions, W=500 per partition], G=8 floats
  * 6 W-chunks (120/120/120/80/40/20 -- tapered so the post-DMA tail is
    tiny), each ONE SWDGE DMA (host packs [P, C*wc*G] contiguous per
    partition; per-partition line 6.4-38.4 KB, above the 4 KB packet
    knee) that casts f32 -> float8e4 in flight.  HBM reads stay f32
    (20 MB/core: the memory-roofline term, ~57.6us at 358 GB/s); fp8
    quarters the DVE select cost via int32 views.
  * descriptor gens live on GpSimd (casting DMAs are SWDGE-only); only
    8 gens total (mask + 6 data + out), all issued up-front -- the
    ~6-deep SWDGE FIFO back-pressures GpSimd harmlessly.
  * select: class-0 copy (int16 view, DVE 4x mode) + 9 copy_predicated
    (int32 views: copy_predicated has no DVE fast modes, so cost = elem
    count -> 4 fp8 per elem is the cheapest legal view; q row stride is
    20 B so int32 elems stay 4-byte aligned).  'Ignored' rows keep
    class-0 garbage which the mask columns annihilate downstream.
  * log on ACT as ln(x + 1e-5): the bias rescues values that e4m3
    flushes to zero.  L row-sum, t = L - log sel, q[:,:,8:16] = sel*t
    on DVE; chunk k's chain is interleaved into chunk k+1's selects.
  * per-class masked sums via TensorE one-hot matmuls (block-diagonal
    trick, grp=10 sample slots per matmul): all 50 groups accumulate
    into one PSUM tile [100, 200]; the fp8 mask (exact 0/1) is built
    from the int8 mask on the early-idle ACT engine.
  * host extracts the 10 diagonal [10, 20] blocks, sums over blocks +
    cores, and finishes the tiny [C]-sized arithmetic in float64.
"""

import numpy as np

import concourse.bacc as bacc
import concourse.tile as tile
from concourse import mybir
from concourse import bass_utils

F32 = mybir.dt.float32
BF16 = mybir.dt.bfloat16
I8 = mybir.dt.int8
I16 = mybir.dt.int16
I32 = mybir.dt.int32

FP8 = True                    # activations as float8e4 (e4m3)
VDT = mybir.dt.float8e4 if FP8 else BF16
LN_BIAS = 1e-5 if FP8 else 0.0   # ln(x + bias): rescue e4m3-flushed zeros

C, G = 10, 8
N_FULL = 500000
NCORES = 8

NS = N_FULL // NCORES  # 62500
P = 125
W = NS // P            # 500
CHUNKS = (120, 120, 120, 80, 40, 20)
GRP = 10
NQCOL = 20             # q columns: sel(8) | sel*t(8) | ones(2) | pad(2)
                       # 20 B sample stride keeps int32 views 4-B aligned

assert sum(CHUNKS) == W and all(wc % GRP == 0 for wc in CHUNKS)
OFFS = tuple(int(np.cumsum((0,) + CHUNKS)[k]) for k in range(len(CHUNKS)))


def build_nc(debug=False):
    """Build the per-core Bass program."""
    p, w, grp = P, W, GRP
    mq = grp * C            # psum partitions (<=128)
    nq = grp * NQCOL        # psum free (<=512 f32)
    assert mq <= 128 and nq <= 512
    nchunk = len(CHUNKS)

    nc = bacc.Bacc("TRN2", target_bir_lowering=False, debug=debug)

    # host packs chunk k as [p, C*wc*G] f32: one contiguous-per-partition
    # DRAM block per chunk -> ONE descriptor gen moves all 10 classes
    a_dr = [
        nc.dram_tensor(f"a{k}", [p, C * wc * G], F32, kind="ExternalInput")
        for k, wc in enumerate(CHUNKS)
    ]
    mi8 = nc.dram_tensor("mi8", [p, w, C], I8, kind="ExternalInput")
    out = nc.dram_tensor("out", [mq, nq], F32, kind="ExternalOutput")

    with tile.TileContext(nc) as tc:
        with (
            tc.tile_pool(name="labp", bufs=1) as labp,
            tc.tile_pool(name="ap", bufs=1) as apool,
            tc.tile_pool(name="qp", bufs=1) as qp,
            tc.tile_pool(name="logp", bufs=1) as logp,
            tc.tile_pool(name="outp", bufs=1) as outp,
            tc.tile_pool(name="psum", bufs=1, space="PSUM") as psump,
        ):
            # ln-bias constant (per-partition scalar for ACT)
            lnb = labp.tile([p, 1], F32)
            nc.gpsimd.memset(lnb[:], LN_BIAS)
            mask_i = labp.tile([p, w, C], I8)
            mask_bf = labp.tile([p, w, C], VDT)

            psum = psump.tile([mq, nq], F32)

            a_t, q_t, logsel_t, l_t, t_t = {}, {}, {}, {}, {}
            for k, wc in enumerate(CHUNKS):
                a_t[k] = apool.tile([p, C, wc, G], VDT, tag=f"a{k}",
                                    name=f"a{k}")
                q_t[k] = qp.tile([p, wc, NQCOL], VDT, tag=f"q{k}",
                                 name=f"q{k}")
                logsel_t[k] = logp.tile([p, wc, G], F32, tag=f"log{k}",
                                        name=f"log{k}")
                l_t[k] = logp.tile([p, wc], F32, tag=f"L{k}", name=f"L{k}")
                t_t[k] = logp.tile([p, wc, G], VDT, tag=f"t{k}",
                                   name=f"t{k}")

            def gen(k):
                nc.gpsimd.dma_start(out=a_t[k][:], in_=a_dr[k].ap())

            def sel(k, c):
                wc, q = CHUNKS[k], q_t[k]
                src = a_t[k][:, c]
                dst = q[:, :, 0:G]
                if c == 0:
                    # plain copy: int16 view hits the DVE 4x_2p fast mode
                    nc.vector.tensor_copy(out=dst.bitcast(I16),
                                          in_=src.bitcast(I16))
                else:
                    # copy_predicated has no fast modes: cost = elem count,
                    # so use the widest legal view (int32 = 4 fp8)
                    nc.vector.copy_predicated(
                        dst.bitcast(I32),
                        mask_i[:, OFFS[k]:OFFS[k] + wc, c:c + 1]
                        .broadcast_to([p, wc, G // 4]),
                        src.bitcast(I32),
                    )

            def counts(k):
                nc.scalar.activation(
                    out=q_t[k][:, :, 2 * G:2 * G + 2],
                    in_=mask_i[:, OFFS[k]:OFFS[k] + CHUNKS[k], 0:2],
                    func=mybir.ActivationFunctionType.Copy,
                    bias=1.0, scale=0.0,
                )

            def pads(k):
                # zero the 2 pad columns so fp8 NaN garbage can't reach psum
                nc.scalar.activation(
                    out=q_t[k][:, :, 2 * G + 2:NQCOL],
                    in_=mask_i[:, OFFS[k]:OFFS[k] + CHUNKS[k], 0:2],
                    func=mybir.ActivationFunctionType.Copy,
                    bias=0.0, scale=0.0,
                )

            def ln(k):
                nc.scalar.activation(
                    out=logsel_t[k][:], in_=q_t[k][:, :, 0:G],
                    func=mybir.ActivationFunctionType.Ln,
                    bias=lnb[:],
                )

            def red(k):
                nc.vector.reduce_sum(
                    out=l_t[k][:], in_=logsel_t[k][:],
                    axis=mybir.AxisListType.X,
                )

            def sub(k):
                nc.vector.tensor_sub(
                    t_t[k][:],
                    l_t[k][:, :, None].broadcast_to([p, CHUNKS[k], G]),
                    logsel_t[k][:],
                )

            def mul(k, engine):
                q = q_t[k]
                engine.tensor_mul(q[:, :, G:2 * G], q[:, :, 0:G], t_t[k][:])

            def mm(k):
                wc, q = CHUNKS[k], q_t[k]
                for gi in range(wc // grp):
                    w0 = OFFS[k] + gi * grp
                    nc.tensor.matmul(
                        psum[:],
                        lhsT=mask_bf[:, w0:w0 + grp, :],
                        rhs=q[:, gi * grp:(gi + 1) * grp, :],
                        start=(k == 0 and gi == 0),
                        stop=(k == nchunk - 1 and gi == wc // grp - 1),
                    )

            # ---- issue order ------------------------------------------------
            # all descriptor gens up-front: mask first (predicates gate every
            # select), then the 6 data chunks.  The SWDGE FIFO back-pressures
            # GpSimd once full; GpSimd has nothing else to do.
            nc.gpsimd.dma_start(out=mask_i[:], in_=mi8.ap())
            for k in range(nchunk):
                gen(k)
            # matmul mask (0/1, exact in any float dtype) on early-idle ACT
            nc.scalar.copy(out=mask_bf[:], in_=mask_i[:])
            # chunk k's post-chain (red -> sub -> mul -> mm) is interleaved
            # into chunk k+1's selects to absorb DVE idle slots
            for k in range(nchunk):
                for c in range(C):
                    sel(k, c)
                    if k > 0:
                        if c == 2:
                            red(k - 1)
                        elif c == 4:
                            sub(k - 1)
                        elif c == 6:
                            mul(k - 1, nc.vector)
                        elif c == 7:
                            mm(k - 1)
                counts(k)
                pads(k)
                ln(k)
            k = nchunk - 1
            red(k)
            sub(k)
            mul(k, nc.vector)
            mm(k)

            out_sb = outp.tile([mq, nq], F32)
            nc.scalar.copy(out=out_sb[:], in_=psum[:])
            nc.sync.dma_start(out=out.ap(), in_=out_sb[:])

    nc.compile()
    return nc


_NC_CACHE = {}


def _get_nc():
    if "full" not in _NC_CACHE:
        _NC_CACHE["full"] = build_nc()
    return _NC_CACHE["full"]


def _reduce_host(outs, grp=GRP):
    """outs: list of per-core [grp*C, grp*NQCOL] partial-sum matrices."""
    total = np.zeros_like(outs[0], dtype=np.float64)
    for o in outs:
        total += o.astype(np.float64)
    agg = np.zeros((C, NQCOL), np.float64)
    for s in range(grp):
        agg += total[s * C:(s + 1) * C, s * NQCOL:(s + 1) * NQCOL]
    S = agg[:, 0:G]
    B = agg[:, G:2 * G]          # sum sel*(L - logsel)
    cnt = agg[:, 2 * G]
    valid = cnt >= 1.5
    with np.errstate(divide="ignore", invalid="ignore"):
        per_class = (B / S).sum(1) - (G - 1) * np.log(S).sum(1)
    num = np.where(valid, per_class, 0.0).sum()
    den = valid.sum() * G * (G - 1)
    return np.array(num / den, dtype=np.float32)


def _run(group_act, target_labels, **spmd_kwargs):
    group_act = np.asarray(group_act, dtype=np.float32)
    labi = np.asarray(target_labels).astype(np.int32) - 1  # -1 => ignored

    in_maps = []
    for k in range(NCORES):
        sl = slice(k * NS, (k + 1) * NS)
        onehot = (labi[sl].reshape(P, W, 1) ==
                  np.arange(C, dtype=np.int32)).astype(np.int8)
        im = {"mi8": onehot}
        ga = (group_act[:, sl, :].reshape(C, P, W, G)
              .transpose(1, 0, 2, 3))                      # [P, C, W, G]
        for ck, wc in enumerate(CHUNKS):
            blk = ga[:, :, OFFS[ck]:OFFS[ck] + wc, :]      # [P, C, wc, G]
            im[f"a{ck}"] = np.ascontiguousarray(blk).reshape(P, C * wc * G)
        in_maps.append(im)

    nc = _get_nc()
    res = bass_utils.run_bass_kernel_spmd(
        nc, in_maps, core_ids=list(range(NCORES)), **spmd_kwargs
    )
    outs = [r["out"] for r in res.results]
    return _reduce_host(outs), res


def kernel(group_act, target_labels):
    return _run(group_act, target_labels)[0]


# revision 9
# speedup vs baseline: 1.1268x; 1.1268x over previous
"""Trainium2 Bass kernel for nn_CrossEntGroup.

Reference computation (see problem):
    labels = target_labels - 1                      # -1 => ignored
    per class c: mask rows with label==c, col_sum S[c,g], p = Am/S,
    M[c,i,j] = sum_n p[n,i] log p[n,j],  loss = mean over valid classes of
    sum_{i!=j} M[c,i,j] / (G*(G-1))

Algebraic reduction used here (single pass over the data):
    sel[n,:]  = group_act[label[n], n, :]       (selected row)
    L[n]      = sum_j log sel[n,j]
    S[c,i]    = sum_{n in c} sel[n,i]
    B[c,i]    = sum_{n in c} sel[n,i] * (L[n] - log sel[n,i])
    per_class[c] = sum_i B[c,i]/S[c,i] - (G-1) * sum_i log S[c,i]
    out = sum_valid per_class / (n_valid * G * (G-1))

Device strategy (per core, N sharded 8 ways -> NS=62500 samples):
  * samples laid out [P=125 partitions, W=500 per partition], G=8 floats
  * 5 tapered sample chunks (170/150/110/50/20 -- large first, small
    last to shrink the post-DMA tail), each fed by 5 class-PAIR SWDGE
    DMAs that cast f32 -> float8e4 in flight.  HBM reads stay f32
    (20 MB/core, the ~57.6us memory-roofline term); fp8 halves the
    DVE select cost (copy_predicated is byte-bound at ~0.7ns/B).
  * the select sweep (class-0 copy + 9 copy_predicated on int16 views)
    is DVE's only bulk job (~36us < stream).  'Ignored' rows keep
    class-0 garbage which the mask columns annihilate downstream.
  * chunk 0-1 post-chains (L row-sum, t = L - log sel, sel*t) run on
    GpSimd (tensor ops at 0.42-0.6 eff), interleaved BETWEEN descriptor
    gens so the SWDGE pushes stay ahead of the stream; chunk 2-4 chains
    stay on DVE, slotted into later sweeps' DMA-pacing bubbles.
  * log on ACT as ln(x + 1e-5): the bias rescues values that e4m3
    flushes to zero (log 0 would poison the sums).
  * per-class masked sums via TensorE one-hot matmuls (block-diagonal
    trick, grp=10 sample slots per matmul): all 50 groups accumulate
    into one PSUM tile [100, 180]; the fp8 mask (exact 0/1) is built
    from the int8 mask on the early-idle ACT engine.
  * host extracts the 10 diagonal [10, 18] blocks, sums over blocks +
    cores, and finishes the tiny [C]-sized arithmetic in float64.
"""

import numpy as np

import concourse.bacc as bacc
import concourse.tile as tile
from concourse import mybir
from concourse import bass_utils

F32 = mybir.dt.float32
BF16 = mybir.dt.bfloat16
I8 = mybir.dt.int8
I16 = mybir.dt.int16

FP8 = True                    # activations as float8e4 (e4m3)
VDT = mybir.dt.float8e4 if FP8 else BF16
LN_BIAS = 1e-5 if FP8 else 0.0   # ln(x + bias): rescue e4m3-flushed zeros

C, G = 10, 8
N_FULL = 500000
NCORES = 8

NS = N_FULL // NCORES  # 62500
P = 125
W = NS // P            # 500
CHUNKS = (170, 150, 110, 50, 20)
GRP = 10
NQCOL = 18             # q columns: sel(8) | sel*t(8) | ones(2)
NPAIR = C // 2
N_GP_CHAIN = 0         # GpSimd fp8 tensor ops are numerically wrong on HW:
                       # keep every chain op on DVE

assert sum(CHUNKS) == W and all(wc % GRP == 0 for wc in CHUNKS)
OFFS = tuple(int(np.cumsum((0,) + CHUNKS)[k]) for k in range(len(CHUNKS)))


def build_nc(debug=False):
    """Build the per-core Bass program."""
    p, w, grp = P, W, GRP
    mq = grp * C            # psum partitions (<=128)
    nq = grp * NQCOL        # psum free (<=512 f32)
    assert mq <= 128 and nq <= 512
    nchunk = len(CHUNKS)

    nc = bacc.Bacc("TRN2", target_bir_lowering=False, debug=debug)

    # host packs chunk k as [pair, p, 2*wc*G]: each (chunk, class-pair)
    # transfer is one contiguous-per-partition DRAM block
    a_dr = [
        nc.dram_tensor(f"a{k}", [NPAIR, p, 2 * wc * G], F32,
                       kind="ExternalInput")
        for k, wc in enumerate(CHUNKS)
    ]
    mi8 = nc.dram_tensor("mi8", [p, w, C], I8, kind="ExternalInput")
    out = nc.dram_tensor("out", [mq, nq], F32, kind="ExternalOutput")

    with tile.TileContext(nc) as tc:
        with (
            tc.tile_pool(name="labp", bufs=1) as labp,
            tc.tile_pool(name="ap", bufs=3) as apool,
            tc.tile_pool(name="qp", bufs=1) as qp,
            tc.tile_pool(name="logp", bufs=1) as logp,
            tc.tile_pool(name="outp", bufs=1) as outp,
            tc.tile_pool(name="psum", bufs=1, space="PSUM") as psump,
        ):
            # ln-bias constant (per-partition scalar for ACT)
            lnb = labp.tile([p, 1], F32)
            nc.gpsimd.memset(lnb[:], LN_BIAS)
            mask_i = labp.tile([p, w, C], I8)
            mask_bf = labp.tile([p, w, C], VDT)

            psum = psump.tile([mq, nq], F32)

            q_t, logsel_t, l_t, t_t = {}, {}, {}, {}
            for k, wc in enumerate(CHUNKS):
                q_t[k] = qp.tile([p, wc, NQCOL], VDT, tag=f"q{k}",
                                 name=f"q{k}")
                logsel_t[k] = logp.tile([p, wc, G], F32, tag=f"log{k}",
                                        name=f"log{k}")
                l_t[k] = logp.tile([p, wc], F32, tag=f"L{k}", name=f"L{k}")
                t_t[k] = logp.tile([p, wc, G], VDT, tag=f"t{k}",
                                   name=f"t{k}")

            a_t = {}

            def gen(k, j):
                # all 5 pair tiles of a chunk live simultaneously: WAR
                # throttling here couples the DMA to DVE progress and
                # measurably starves the queue
                t = apool.tile([p, 2, CHUNKS[k], G], VDT, tag=f"a{k}",
                               name=f"a{k}_{j}", bufs=NPAIR)
                nc.gpsimd.dma_start(out=t[:], in_=a_dr[k].ap()[j])
                a_t[(k, j)] = t

            NV = G // 2 if FP8 else G   # int16-view elems per sample

            def sel(k, c):
                wc, q = CHUNKS[k], q_t[k]
                src = a_t[(k, c // 2)][:, c % 2]
                dst = q[:, :, 0:G]
                if FP8:
                    src = src.bitcast(I16)
                    dst = dst.bitcast(I16)
                if c == 0:
                    nc.vector.tensor_copy(out=dst, in_=src)
                else:
                    nc.vector.copy_predicated(
                        dst,
                        mask_i[:, OFFS[k]:OFFS[k] + wc, c:c + 1]
                        .broadcast_to([p, wc, NV]),
                        src,
                    )

            def counts(k):
                nc.scalar.activation(
                    out=q_t[k][:, :, 2 * G:NQCOL],
                    in_=mask_i[:, OFFS[k]:OFFS[k] + CHUNKS[k], 0:2],
                    func=mybir.ActivationFunctionType.Copy,
                    bias=1.0, scale=0.0,
                )

            def ln(k):
                nc.scalar.activation(
                    out=logsel_t[k][:], in_=q_t[k][:, :, 0:G],
                    func=mybir.ActivationFunctionType.Ln,
                    bias=lnb[:],
                )

            def red(k, eng):
                eng.reduce_sum(
                    out=l_t[k][:], in_=logsel_t[k][:],
                    axis=mybir.AxisListType.X,
                )

            def sub(k, eng):
                eng.tensor_sub(
                    t_t[k][:],
                    l_t[k][:, :, None].broadcast_to([p, CHUNKS[k], G]),
                    logsel_t[k][:],
                )

            def mul(k, eng):
                q = q_t[k]
                eng.tensor_mul(q[:, :, G:2 * G], q[:, :, 0:G], t_t[k][:])

            def chain_gp(k):
                # GpSimd reduce only supports the partition axis and its
                # tensor_tensor mishandles stride-0 broadcast operands, so
                # red/sub stay on DVE (slotted into the next sweep); only
                # the plain packed multiply runs here
                mul(k, nc.gpsimd)

            def mm(k):
                wc, q = CHUNKS[k], q_t[k]
                for gi in range(wc // grp):
                    w0 = OFFS[k] + gi * grp
                    nc.tensor.matmul(
                        psum[:],
                        lhsT=mask_bf[:, w0:w0 + grp, :],
                        rhs=q[:, gi * grp:(gi + 1) * grp, :],
                        start=(k == 0 and gi == 0),
                        stop=(k == nchunk - 1 and gi == wc // grp - 1),
                    )

            # ---- software-pipelined issue order -------------------------
            # GpSimd program order alternates descriptor-gen batches with
            # chunk 0/1 chains so pushes stay ahead of the 361 GB/s FIFO
            # drain while the chains still run decades before the tail.
            gen(0, 0)
            nc.gpsimd.dma_start(out=mask_i[:], in_=mi8.ap())
            # matmul mask (0/1, exact in any float dtype) on early-idle ACT
            nc.scalar.copy(out=mask_bf[:], in_=mask_i[:])
            for j in range(1, NPAIR):
                gen(0, j)
            for j in range(NPAIR):
                gen(1, j)
            for k in range(nchunk):
                for c in range(C):
                    sel(k, c)
                    # DVE chain pieces slotted into this sweep's DMA-pacing
                    # bubbles: the row-sum always runs here; sub/mul only
                    # for chunks whose chain is not on GpSimd
                    if k > 0:
                        if c == 2:
                            red(k - 1, nc.vector)
                        elif c == 4:
                            sub(k - 1, nc.vector)
                        elif c == 6 and k > N_GP_CHAIN:
                            mul(k - 1, nc.vector)
                    if c == 7 and k > 0:
                        mm(k - 1)
                counts(k)
                ln(k)
                # GpSimd: next gen batch FIRST (descriptor pushes must stay
                # ahead of the FIFO drain), then chunk k's chain (k <
                # N_GP_CHAIN) which stalls on ln(k) harmlessly
                if k + 2 < nchunk:
                    for j in range(NPAIR):
                        gen(k + 2, j)
                if k < N_GP_CHAIN:
                    chain_gp(k)
            k = nchunk - 1
            red(k, nc.vector)
            sub(k, nc.vector)
            mul(k, nc.vector)
            mm(k)

            out_sb = outp.tile([mq, nq], F32)
            nc.scalar.copy(out=out_sb[:], in_=psum[:])
            nc.sync.dma_start(out=out.ap(), in_=out_sb[:])

    nc.compile()
    return nc


_NC_CACHE = {}


def _get_nc():
    if "full" not in _NC_CACHE:
        _NC_CACHE["full"] = build_nc()
    return _NC_CACHE["full"]


def _reduce_host(outs, grp=GRP):
    """outs: list of per-core [grp*C, grp*NQCOL] partial-sum matrices."""
    total = np.zeros_like(outs[0], dtype=np.float64)
    for o in outs:
        total += o.astype(np.float64)
    agg = np.zeros((C, NQCOL), np.float64)
    for s in range(grp):
        agg += total[s * C:(s + 1) * C, s * NQCOL:(s + 1) * NQCOL]
    S = agg[:, 0:G]
    B = agg[:, G:2 * G]          # sum sel*(L - logsel)
    cnt = agg[:, 2 * G]
    valid = cnt >= 1.5
    with np.errstate(divide="ignore", invalid="ignore"):
        per_class = (B / S).sum(1) - (G - 1) * np.log(S).sum(1)
    num = np.where(valid, per_class, 0.0).sum()
    den = valid.sum() * G * (G - 1)
    return np.array(num / den, dtype=np.float32)


def _run(group_act, target_labels, **spmd_kwargs):
    group_act = np.asarray(group_act, dtype=np.float32)
    labi = np.asarray(target_labels).astype(np.int32) - 1  # -1 => ignored

    in_maps = []
    for k in range(NCORES):
        sl = slice(k * NS, (k + 1) * NS)
        onehot = (labi[sl].reshape(P, W, 1) ==
                  np.arange(C, dtype=np.int32)).astype(np.int8)
        im = {"mi8": onehot}
        ga = group_act[:, sl, :].reshape(C, P, W, G)
        for ck, wc in enumerate(CHUNKS):
            blk = ga[:, :, OFFS[ck]:OFFS[ck] + wc, :]          # [C,P,wc,G]
            blk = (blk.reshape(NPAIR, 2, P, wc, G)
                      .transpose(0, 2, 1, 3, 4)
                      .reshape(NPAIR, P, 2 * wc * G))
            im[f"a{ck}"] = np.ascontiguousarray(blk)
        in_maps.append(im)

    nc = _get_nc()
    res = bass_utils.run_bass_kernel_spmd(
        nc, in_maps, core_ids=list(range(NCORES)), **spmd_kwargs
    )
    outs = [r["out"] for r in res.results]
    return _reduce_host(outs), res


def kernel(group_act, target_labels):
    return _run(group_act, target_labels)[0]


# revision 11
# speedup vs baseline: 1.1416x; 1.0132x over previous
"""Trainium2 Bass kernel for nn_CrossEntGroup.

Reference computation (see problem):
    labels = target_labels - 1                      # -1 => ignored
    per class c: mask rows with label==c, col_sum S[c,g], p = Am/S,
    M[c,i,j] = sum_n p[n,i] log p[n,j],  loss = mean over valid classes of
    sum_{i!=j} M[c,i,j] / (G*(G-1))

Algebraic reduction used here (single pass over the data):
    sel[n,:]  = group_act[label[n], n, :]       (selected row)
    L[n]      = sum_j log sel[n,j]
    S[c,i]    = sum_{n in c} sel[n,i]
    B[c,i]    = sum_{n in c} sel[n,i] * (L[n] - log sel[n,i])
    per_class[c] = sum_i B[c,i]/S[c,i] - (G-1) * sum_i log S[c,i]
    out = sum_valid per_class / (n_valid * G * (G-1))

Device strategy (per core, N sharded 8 ways -> NS=62500 samples):
  * samples laid out [P=125 partitions, W=500 per partition], G=8 floats
  * 4 sample chunks (170/150/140/40).  Chunks 0-2 stream as 5 class-PAIR
    SWDGE DMAs each (per-partition lines 10.9/9.6/9.0 KB, above the
    4 KB packet knee); the small last chunk streams as 2 class-HALF
    DMAs (6.4 KB lines) so it arrives at full rate from just 2
    descriptor gens while still gating the DVE incrementally.  All DMAs
    cast f32 -> fp8e4 in flight; HBM reads stay f32 (20 MB/core, the
    ~57.6 us roofline term).  17 gens total keeps the ~6-deep SWDGE
    FIFO from ever gen-rate-limiting the stream tail (the failure mode
    of finer tapers: gen ~0.8 us > small-transfer time).
  * select sweep: class-0 copy + 9 copy_predicated (int16 views) on
    DVE -- byte-bound at ~0.7 ns/B/lane, the dominant DVE cost.
    'Ignored' rows keep class-0 garbage which the mask columns
    annihilate downstream.  GpSimd fp8 tensor ops are numerically
    wrong on HW, so every chain op stays on DVE.
  * log on ACT as ln(x + 1e-5) (bias rescues e4m3-flushed zeros),
    issued BEFORE counts so the serial red->sub->mul tail chain starts
    sooner.  Chunk k-1's chain is slotted into sweep k's DMA-pacing
    bubbles; chunk 2's chain runs in the idle gap before the last
    chunk's data lands.
  * per-class masked sums via TensorE one-hot matmuls (block-diagonal
    trick, grp=10 sample slots per matmul): all 50 groups accumulate
    into one PSUM tile [100, 180]; the fp8 mask (exact 0/1) is built
    from the int8 mask on the early-idle ACT engine.
  * host extracts the 10 diagonal [10, 18] blocks, sums over blocks +
    cores, and finishes the tiny [C]-sized arithmetic in float64.
"""

import numpy as np

import concourse.bacc as bacc
import concourse.tile as tile
from concourse import mybir
from concourse import bass_utils

F32 = mybir.dt.float32
BF16 = mybir.dt.bfloat16
I8 = mybir.dt.int8
I16 = mybir.dt.int16

FP8 = True                    # activations as float8e4 (e4m3)
VDT = mybir.dt.float8e4 if FP8 else BF16
LN_BIAS = 1e-5 if FP8 else 0.0   # ln(x + bias): rescue e4m3-flushed zeros

C, G = 10, 8
N_FULL = 500000
NCORES = 8

NS = N_FULL // NCORES  # 62500
P = 125
W = NS // P            # 500
CHUNKS = (170, 150, 140, 40)
GRP = 10
NQCOL = 18             # q columns: sel(8) | sel*t(8) | ones(2)
NPAIR = C // 2
NHALF = C // 5         # 2 class-half DMAs for the last chunk

assert sum(CHUNKS) == W and all(wc % GRP == 0 for wc in CHUNKS)
OFFS = tuple(int(np.cumsum((0,) + CHUNKS)[k]) for k in range(len(CHUNKS)))
LASTK = len(CHUNKS) - 1


def build_nc(debug=False):
    """Build the per-core Bass program."""
    p, w, grp = P, W, GRP
    mq = grp * C            # psum partitions (<=128)
    nq = grp * NQCOL        # psum free (<=512 f32)
    assert mq <= 128 and nq <= 512
    nchunk = len(CHUNKS)

    nc = bacc.Bacc("TRN2", target_bir_lowering=False, debug=debug)

    # host packs chunks 0..2 as [pair, p, 2*wc*G], the last chunk as
    # [half, p, 5*wc*G]: every transfer is contiguous-per-partition
    a_dr = [
        nc.dram_tensor(f"a{k}",
                       [NHALF, p, 5 * CHUNKS[k] * G] if k == LASTK
                       else [NPAIR, p, 2 * CHUNKS[k] * G],
                       F32, kind="ExternalInput")
        for k in range(nchunk)
    ]
    mi8 = nc.dram_tensor("mi8", [p, w, C], I8, kind="ExternalInput")
    out = nc.dram_tensor("out", [mq, nq], F32, kind="ExternalOutput")

    with tile.TileContext(nc) as tc:
        with (
            tc.tile_pool(name="labp", bufs=1) as labp,
            tc.tile_pool(name="ap", bufs=3) as apool,
            tc.tile_pool(name="qp", bufs=1) as qp,
            tc.tile_pool(name="logp", bufs=1) as logp,
            tc.tile_pool(name="outp", bufs=1) as outp,
            tc.tile_pool(name="psum", bufs=1, space="PSUM") as psump,
        ):
            # ln-bias constant (per-partition scalar for ACT)
            lnb = labp.tile([p, 1], F32)
            nc.gpsimd.memset(lnb[:], LN_BIAS)
            mask_i = labp.tile([p, w, C], I8)
            mask_bf = labp.tile([p, w, C], VDT)

            psum = psump.tile([mq, nq], F32)

            q_t, logsel_t, l_t, t_t = {}, {}, {}, {}
            for k, wc in enumerate(CHUNKS):
                q_t[k] = qp.tile([p, wc, NQCOL], VDT, tag=f"q{k}",
                                 name=f"q{k}")
                logsel_t[k] = logp.tile([p, wc, G], F32, tag=f"log{k}",
                                        name=f"log{k}")
                l_t[k] = logp.tile([p, wc], F32, tag=f"L{k}", name=f"L{k}")
                t_t[k] = logp.tile([p, wc, G], VDT, tag=f"t{k}",
                                   name=f"t{k}")

            a_t = {}

            def gen(k, j):
                # all tiles of a chunk live simultaneously: WAR throttling
                # here couples the DMA to DVE progress and starves the queue
                ncls = 5 if k == LASTK else 2
                nbuf = NHALF if k == LASTK else NPAIR
                t = apool.tile([p, ncls, CHUNKS[k], G], VDT, tag=f"a{k}",
                               name=f"a{k}_{j}", bufs=nbuf)
                nc.gpsimd.dma_start(out=t[:], in_=a_dr[k].ap()[j])
                a_t[(k, j)] = t

            NV = G // 2 if FP8 else G   # int16-view elems per sample

            def sel(k, c):
                wc, q = CHUNKS[k], q_t[k]
                ncls = 5 if k == LASTK else 2
                src = a_t[(k, c // ncls)][:, c % ncls]
                dst = q[:, :, 0:G]
                if FP8:
                    src = src.bitcast(I16)
                    dst = dst.bitcast(I16)
                if c == 0:
                    nc.vector.tensor_copy(out=dst, in_=src)
                else:
                    nc.vector.copy_predicated(
                        dst,
                        mask_i[:, OFFS[k]:OFFS[k] + wc, c:c + 1]
                        .broadcast_to([p, wc, NV]),
                        src,
                    )

            def counts(k):
                nc.scalar.activation(
                    out=q_t[k][:, :, 2 * G:NQCOL],
                    in_=mask_i[:, OFFS[k]:OFFS[k] + CHUNKS[k], 0:2],
                    func=mybir.ActivationFunctionType.Copy,
                    bias=1.0, scale=0.0,
                )

            def ln(k):
                nc.scalar.activation(
                    out=logsel_t[k][:], in_=q_t[k][:, :, 0:G],
                    func=mybir.ActivationFunctionType.Ln,
                    bias=lnb[:],
                )

            def red(k):
                nc.vector.reduce_sum(
                    out=l_t[k][:], in_=logsel_t[k][:],
                    axis=mybir.AxisListType.X,
                )

            def sub(k):
                nc.vector.tensor_sub(
                    t_t[k][:],
                    l_t[k][:, :, None].broadcast_to([p, CHUNKS[k], G]),
                    logsel_t[k][:],
                )

            def mul(k):
                q = q_t[k]
                nc.vector.tensor_mul(q[:, :, G:2 * G], q[:, :, 0:G],
                                     t_t[k][:])

            def mm(k):
                wc, q = CHUNKS[k], q_t[k]
                for gi in range(wc // grp):
                    w0 = OFFS[k] + gi * grp
                    nc.tensor.matmul(
                        psum[:],
                        lhsT=mask_bf[:, w0:w0 + grp, :],
                        rhs=q[:, gi * grp:(gi + 1) * grp, :],
                        start=(k == 0 and gi == 0),
                        stop=(k == nchunk - 1 and gi == wc // grp - 1),
                    )

            # ---- software-pipelined issue order -------------------------
            gen(0, 0)
            nc.gpsimd.dma_start(out=mask_i[:], in_=mi8.ap())
            # matmul mask (0/1, exact in any float dtype) on early-idle ACT
            nc.scalar.copy(out=mask_bf[:], in_=mask_i[:])
            for j in range(1, NPAIR):
                gen(0, j)
            for j in range(NPAIR):
                gen(1, j)
            for k in range(nchunk):
                if k == LASTK:
                    # chunk 2's chain fills the DVE idle gap before the
                    # last chunk's data lands
                    red(k - 1)
                    sub(k - 1)
                    mul(k - 1)
                for c in range(C):
                    sel(k, c)
                    if 0 < k < LASTK:
                        if c == 2:
                            red(k - 1)
                        elif c == 4:
                            sub(k - 1)
                        elif c == 6:
                            mul(k - 1)
                    if c == 7 and k > 0:
                        mm(k - 1)
                ln(k)       # before counts: ln gates the serial tail chain
                counts(k)
                nxt = k + 2
                if nxt < nchunk:
                    for j in range(NHALF if nxt == LASTK else NPAIR):
                        gen(nxt, j)
            k = nchunk - 1
            red(k)
            sub(k)
            mul(k)
            mm(k)

            out_sb = outp.tile([mq, nq], F32)
            nc.scalar.copy(out=out_sb[:], in_=psum[:])
            nc.sync.dma_start(out=out.ap(), in_=out_sb[:])

    nc.compile()
    return nc


_NC_CACHE = {}


def _get_nc():
    if "full" not in _NC_CACHE:
        _NC_CACHE["full"] = build_nc()
    return _NC_CACHE["full"]


def _reduce_host(outs, grp=GRP):
    """outs: list of per-core [grp*C, grp*NQCOL] partial-sum matrices."""
    total = np.zeros_like(outs[0], dtype=np.float64)
    for o in outs:
        total += o.astype(np.float64)
    agg = np.zeros((C, NQCOL), np.float64)
    for s in range(grp):
        agg += total[s * C:(s + 1) * C, s * NQCOL:(s + 1) * NQCOL]
    S = agg[:, 0:G]
    B = agg[:, G:2 * G]          # sum sel*(L - logsel)
    cnt = agg[:, 2 * G]
    valid = cnt >= 1.5
    with np.errstate(divide="ignore", invalid="ignore"):
        per_class = (B / S).sum(1) - (G - 1) * np.log(S).sum(1)
    num = np.where(valid, per_class, 0.0).sum()
    den = valid.sum() * G * (G - 1)
    return np.array(num / den, dtype=np.float32)


def _run(group_act, target_labels, **spmd_kwargs):
    group_act = np.asarray(group_act, dtype=np.float32)
    labi = np.asarray(target_labels).astype(np.int32) - 1  # -1 => ignored

    in_maps = []
    for k in range(NCORES):
        sl = slice(k * NS, (k + 1) * NS)
        onehot = (labi[sl].reshape(P, W, 1) ==
                  np.arange(C, dtype=np.int32)).astype(np.int8)
        im = {"mi8": onehot}
        ga = group_act[:, sl, :].reshape(C, P, W, G)
        for ck, wc in enumerate(CHUNKS):
            blk = ga[:, :, OFFS[ck]:OFFS[ck] + wc, :]          # [C,P,wc,G]
            if ck == LASTK:
                blk = (blk.reshape(NHALF, 5, P, wc, G)
                          .transpose(0, 2, 1, 3, 4)
                          .reshape(NHALF, P, 5 * wc * G))
            else:
                blk = (blk.reshape(NPAIR, 2, P, wc, G)
                          .transpose(0, 2, 1, 3, 4)
                          .reshape(NPAIR, P, 2 * wc * G))
            im[f"a{ck}"] = np.ascontiguousarray(blk)
        in_maps.append(im)

    nc = _get_nc()
    res = bass_utils.run_bass_kernel_spmd(
        nc, in_maps, core_ids=list(range(NCORES)), **spmd_kwargs
    )
    outs = [r["out"] for r in res.results]
    return _reduce_host(outs), res


def kernel(group_act, target_labels):
    return _run(group_act, target_labels)[0]
